# revision 1
# baseline (speedup 1.0000x reference)
"""Causal self-attention (B=4, T=2048, C=1024, H=16) on 8 trn2 NeuronCores.

Sharding: 2 heads per core for QKV+attention (tensor-parallel over heads);
two AllToAlls redistribute per-head attention outputs into per-core row
slices for a row-parallel output projection. QKV projection of batch b+1
is interleaved into the ACT-bound attention stream of batch b so the PE
fills its exp-wait gaps. Host does only layout glue (transpose of x,
weight column gather, final concat).
"""

import math
from contextlib import ExitStack

import numpy as np

NCORES = 8
B, T, C = 4, 2048, 1024
H = 16
D = C // H  # 64
HPC = H // NCORES  # heads per core = 2
BT = B * T  # 8192
ROWS_PER_CORE = BT // NCORES  # 1024
HALF_ROWS = ROWS_PER_CORE // 2  # 512 rows per core per A2A half
NKT = T // 128  # 16 k-tiles per batch
NEG = -1.0e30

_compiled = None


def _build(no_collective=False):
    import concourse.tile as tile
    from concourse import bacc, mybir
    from concourse.masks import make_identity, make_lower_triangular

    f32 = mybir.dt.float32
    f32r = mybir.dt.float32r

    nc = bacc.Bacc()

    # ---- DRAM I/O (per-core views; same kernel on all 8 cores) ----
    xt_d = nc.dram_tensor("xt", [C, BT], f32r, kind="ExternalInput")
    wqkv_d = nc.dram_tensor("wqkv", [C, 3 * 128], f32r, kind="ExternalInput")
    bqkv_d = nc.dram_tensor("bqkv", [128, 3], f32, kind="ExternalInput")
    wp_d = nc.dram_tensor("wp", [C, C], f32r, kind="ExternalInput")
    bp_d = nc.dram_tensor("bp", [1, C], f32, kind="ExternalInput")
    ones_d = nc.dram_tensor("ones", [128, NKT * HPC], f32r, kind="ExternalInput")
    out_d = nc.dram_tensor("out", [ROWS_PER_CORE, C], f32, kind="ExternalOutput")

    # internal DRAM for the four collectives (one per batch; shard = 256 rows)
    QROWS = T // NCORES  # 256
    y_loc = [nc.dram_tensor(f"y_loc{q}", [NCORES, 128, QROWS], f32r) for q in range(B)]
    y_all = [nc.dram_tensor(f"y_all{q}", [NCORES, 128, QROWS], f32r) for q in range(B)]

    xt_r = xt_d[:, :].rearrange("(j p) t -> p j t", p=128)  # [128, 8, BT]
    wqkv_r = wqkv_d[:, :].rearrange("(j p) f -> p j f", p=128)  # [128, 8, 384]
    wp_r = wp_d[:, :].rearrange("(j p) f -> p j f", p=128)  # [128, 8, 1024]

    with tile.TileContext(nc) as tc, ExitStack() as ctx:
        qkv_pool = ctx.enter_context(tc.tile_pool(name="qkv_pool", bufs=2))
        wpool = ctx.enter_context(tc.tile_pool(name="wpool", bufs=1))
        xt_pool = ctx.enter_context(tc.tile_pool(name="xt_pool", bufs=3))
        vtmp_pool = ctx.enter_context(tc.tile_pool(name="vtmp", bufs=1))
        pt_pool = ctx.enter_context(tc.tile_pool(name="pt", bufs=3))
        r_pool = ctx.enter_context(tc.tile_pool(name="rp", bufs=1))
        yt_pool = ctx.enter_context(tc.tile_pool(name="yt", bufs=2))
        ytr_pool = ctx.enter_context(tc.tile_pool(name="ytr", bufs=2))
        out_pool = ctx.enter_context(tc.tile_pool(name="op", bufs=2))
        ya_pool = ctx.enter_context(tc.tile_pool(name="ya", bufs=4))
        ps_big = ctx.enter_context(tc.tile_pool(name="ps_big", bufs=3, space="PSUM"))
        ps_yt = ctx.enter_context(tc.tile_pool(name="ps_yt", bufs=1, space="PSUM"))

        # ---- qkv weights + attention constants (needed from the start) ----
        wq_sb = wpool.tile([128, 8, 3 * 128], f32r)
        bias_sb = wpool.tile([128, 3], f32)

        def emit_wq_load():
            for j in range(8):
                nc.sync.dma_start(out=wq_sb[:, j, :], in_=wqkv_r[:, j, :])
            nc.sync.dma_start(out=bias_sb, in_=bqkv_d[:, :])
        mneg = wpool.tile([128, 128], f32, tag="mneg")
        ident = wpool.tile([128, 128], f32, tag="ident")
        # projection weights are declared now but DMA'd later (emit_wp_load)
        wp_sb = wpool.tile([128, 8, C], f32r)
        bp_row = wpool.tile([128, C], f32, tag="bp_row")
        bias_bc = wpool.tile([128, C], f32, tag="bias_bc")

        def emit_wp_load():
            for j in range(8):
                nc.sync.dma_start(out=wp_sb[:, j, :], in_=wp_r[:, j, :])
            nc.sync.dma_start(out=bp_row[0:1, :], in_=bp_d[:, :])
            nc.gpsimd.partition_broadcast(bias_bc[:, :], bp_row[0:1, :])

        def phase1(b):
            """QKV projection for batch b (generator: yields after each
            (tok-tile, m-chunk) psum group; 12 yields). V layout
            [tok128, 65, ktile, slot]; row 64 = ones."""
            qT = qkv_pool.tile([128, T], f32r, tag="qT", name=f"qT{b}")
            kT = qkv_pool.tile([128, T], f32r, tag="kT", name=f"kT{b}")
            V = qkv_pool.tile([128, D + 1, NKT, HPC], f32r, tag="V", name=f"V{b}")
            nc.gpsimd.dma_start(out=V[:, D, :, :], in_=ones_d[:, :])
            result[b] = (qT, kT, V)

            xt_tiles = {}

            def load_xt(tt):
                tok0 = b * T + tt * 512
                xt_t = xt_pool.tile([128, 8, 512], f32r, tag="xt", name=f"xt{b}_{tt}")
                nc.sync.dma_start(out=xt_t[:, 0:4, :], in_=xt_r[:, 0:4, tok0 : tok0 + 512])
                nc.sync.dma_start(out=xt_t[:, 4:8, :], in_=xt_r[:, 4:8, tok0 : tok0 + 512])
                xt_tiles[tt] = xt_t

            load_xt(0)
            if b == 0:
                emit_wq_load()
            for tt in range(4):  # 512-token tiles
                if tt + 1 < 4:
                    load_xt(tt + 1)  # prefetch one tile ahead
                xt_t = xt_tiles.pop(tt)
                for m in range(3):  # q, k, v feature chunks
                    ps = ps_big.tile([128, 1024], f32, tag="big")
                    for j in range(8):
                        nc.tensor.matmul(
                            ps[:, 0:512],
                            wq_sb[:, j, m * 128 : (m + 1) * 128],
                            xt_t[:, j, :],
                            start=(j == 0),
                            stop=(j == 7),
                        )
                    if m == 0:
                        nc.vector.tensor_scalar_add(
                            qT[:, tt * 512 : (tt + 1) * 512], ps[:, 0:512], bias_sb[:, 0:1]
                        )
                    elif m == 1:
                        nc.vector.tensor_scalar_add(
                            kT[:, tt * 512 : (tt + 1) * 512], ps[:, 0:512], bias_sb[:, 1:2]
                        )
                    else:
                        # v chunk: bias-add to SBUF, then PE-transpose back into
                        # the spare second bank of the same psum slot
                        vt_t = vtmp_pool.tile([128, 512], f32)
                        nc.vector.tensor_scalar_add(vt_t[:, :], ps[:, 0:512], bias_sb[:, 2:3])
                        for i in range(4):
                            pv = ps[:, 512 + i * 128 : 640 + i * 128]
                            nc.tensor.transpose(pv, vt_t[:, i * 128 : (i + 1) * 128], ident[:, :])
                            kt_idx = tt * 4 + i
                            for s in range(HPC):
                                nc.vector.tensor_copy(
                                    V[:, 0:D, kt_idx, s], pv[:, s * D : (s + 1) * D]
                                )
                    yield

        def _emit_av(ps_y, V, s, pt, segs, kt, last):
            for lo, hi in segs:
                nc.tensor.matmul(
                    ps_y[0 : D + 1, lo:hi],
                    V[:, :, kt, s],
                    pt[:, lo:hi],
                    start=(kt == 0),
                    stop=last,
                )

        def phase2(b, s):
            """Causal attention for (batch b, head-slot s). Generator:
            yields after each k-tile strip (24 yields)."""
            qT, kT, V = result[b]
            p0 = s * D  # partition base of this head in qT/kT
            for qs in range(2):  # 1024-wide query supertiles
                ps_y = ps_yt.tile([128, 1024], f32, tag="yt", name="ps_y")
                nkt = 8 * (qs + 1)
                pending = []
                for kt in range(nkt):
                    off = max(0, kt * 128 - qs * 1024)
                    if off < 512:
                        segs = [(off, 512), (512, 1024)]
                    else:
                        segs = [(off, 1024)]
                    ps_s = ps_big.tile([128, 1024], f32, tag="big")
                    for lo, hi in segs:
                        nc.tensor.matmul(
                            ps_s[:, lo:hi],
                            kT[p0 : p0 + D, kt * 128 : (kt + 1) * 128],
                            qT[p0 : p0 + D, qs * 1024 + lo : qs * 1024 + hi],
                            start=True,
                            stop=True,
                        )
                    if kt * 128 >= qs * 1024:  # diagonal strip -> causal mask
                        nc.vector.tensor_add(
                            ps_s[:, off : off + 128], ps_s[:, off : off + 128], mneg[:, :]
                        )
                    pt = pt_pool.tile([128, 1024], f32r)
                    nc.scalar.activation(
                        pt[:, off:1024],
                        ps_s[:, off:1024],
                        mybir.ActivationFunctionType.Exp,
                        scale=1.0 / math.sqrt(D),
                    )
                    pending.append((pt, segs, kt))
                    if len(pending) > 3:  # 3-strip AV skew
                        _emit_av(ps_y, V, s, *pending.pop(0), last=False)
                    yield kt % 3 == 0
                while pending:
                    _emit_av(ps_y, V, s, *pending.pop(0), last=not pending)

                # free the psum accumulator quickly with one copy, then
                # normalize from SBUF off the slot-critical path
                yt_raw = ytr_pool.tile([128, 1024], f32)
                nc.scalar.copy(yt_raw[0:65, 0:512], ps_y[0:65, 0:512])
                nc.vector.tensor_copy(yt_raw[0:65, 512:1024], ps_y[0:65, 512:1024])
                r_t = r_pool.tile([128, 1024], f32, tag="r")
                nc.vector.reciprocal(r_t[0:1, :], yt_raw[64:65, :])
                rb_t = r_pool.tile([128, 1024], f32, tag="rb")
                nc.gpsimd.partition_broadcast(rb_t[0:64, :], r_t[0:1, :])
                yt_sb = yt_pool.tile([128, 1024], f32r)
                nc.vector.tensor_mul(yt_sb[0:64, :], yt_raw[0:64, :], rb_t[0:64, :])
                for piece in range(4):
                    shard = (qs * 1024 + piece * 256) // 256
                    nc.sync.dma_start(
                        out=y_loc[b][shard, p0 : p0 + D, :],
                        in_=yt_sb[0:64, piece * 256 : (piece + 1) * 256],
                    )

        def emit_a2a(q):
            if no_collective:
                return
            nc.gpsimd.collective_compute(
                "AllToAll",
                mybir.AluOpType.bypass,
                replica_groups=[list(range(NCORES))],
                ins=[y_loc[q][:, :, :]],
                outs=[y_all[q][:, :, :]],
            )

        def proj(q):
            """Output projection for this core's 256 rows of batch q.
            Generator: yields after each of 2 row-tiles."""
            y_src = y_loc[q] if no_collective else y_all[q]
            ya_tiles = {}
            for rt in range(2):
                ps_o = ps_big.tile([128, 1024], f32, tag="big")
                for i in range(8):  # feature chunks (source cores)
                    if rt == 0:
                        ya = ya_pool.tile([128, 256], f32r)
                        nc.sync.dma_start(out=ya, in_=y_src[i, :, :])
                        ya_tiles[i] = ya
                    ya = ya_tiles[i][:, rt * 128 : (rt + 1) * 128]
                    for lo, hi in ((0, 512), (512, 1024)):
                        nc.tensor.matmul(
                            ps_o[:, lo:hi],
                            ya,
                            wp_sb[:, i, lo:hi],
                            start=(i == 0),
                            stop=(i == 7),
                        )
                out_sb = out_pool.tile([128, 1024], f32)
                nc.vector.tensor_add(out_sb[:, :], ps_o[:, :], bias_bc[:, :])
                row = q * 256 + rt * 128
                nc.sync.dma_start(out=out_d[row : row + 128, :], in_=out_sb[:, :])
                yield

        def run_interleaved(primary, filler):
            """Drain `primary`, advancing `filler` at hinted insertion points
            (qsuper pipeline warm-up bubbles and every 4th strip)."""
            for hint in primary:
                if filler is not None and hint:
                    try:
                        next(filler)
                    except StopIteration:
                        filler = None
            return filler

        def drain(gen):
            if gen is not None:
                for _ in gen:
                    pass

        def chain(*gens):
            for g in gens:
                if g is not None:
                    yield from g

        result = {}
        # startup: batch 0 qkv stands alone; constants that phase1 itself
        # doesn't need are emitted after its first group to keep the DMA
        # queues clear at kernel start
        p10 = phase1(0)
        next(p10)
        make_identity(nc, ident[:, :])
        make_lower_triangular(nc, mneg[:, :], val=NEG, diag=False)
        drain(p10)
        for b in range(B):
            # filler work for this batch's attention stream: next batch's
            # qkv projection, then the projection of batch b-2 (whose A2A
            # has long completed)
            parts = []
            if b < B - 1:
                parts.append(phase1(b + 1))
            if b >= 2:
                parts.append(proj(b - 2))
            filler = chain(*parts) if parts else None
            for s in range(HPC):
                filler = run_interleaved(phase2(b, s), filler)
            drain(filler)
            if b == 0:
                emit_wp_load()  # off the critical startup path
            emit_a2a(b)
        drain(proj(2))
        drain(proj(3))

    nc.compile()
    return nc


def _get_compiled():
    global _compiled
    if _compiled is None:
        _compiled = _build()
    return _compiled


def _make_in_maps(x, W_attn, b_attn, W_proj, b_proj):
    xt = np.ascontiguousarray(x.reshape(BT, C).T)  # [C, BT]
    bp = np.ascontiguousarray(b_proj.reshape(1, C))
    ones = np.ones((128, NKT * HPC), dtype=np.float32)
    in_maps = []
    for c in range(NCORES):
        heads = [HPC * c + s for s in range(HPC)]
        cols = []
        for m in range(3):  # q, k, v blocks of W_attn
            for h in heads:
                cols.extend(range(m * C + h * D, m * C + (h + 1) * D))
        cols = np.asarray(cols)
        in_maps.append(
            {
                "xt": xt,
                "wqkv": np.ascontiguousarray(W_attn[:, cols]),
                "bqkv": np.ascontiguousarray(b_attn[cols].reshape(3, 128).T),
                "wp": W_proj,
                "bp": bp,
                "ones": ones,
            }
        )
    return in_maps


def kernel(x, W_attn, b_attn, W_proj, b_proj):
    from concourse.bass_utils import run_bass_kernel_spmd

    x = np.asarray(x, dtype=np.float32)
    W_attn = np.asarray(W_attn, dtype=np.float32)
    b_attn = np.asarray(b_attn, dtype=np.float32)
    W_proj = np.asarray(W_proj, dtype=np.float32)
    b_proj = np.asarray(b_proj, dtype=np.float32)

    nc = _get_compiled()
    in_maps = _make_in_maps(x, W_attn, b_attn, W_proj, b_proj)
    res = run_bass_kernel_spmd(nc, in_maps, core_ids=list(range(NCORES)))

    # core c's output: for each batch q, rows [256c, 256c+256) of that batch
    out = np.empty((BT, C), dtype=np.float32)
    for c in range(NCORES):
        o = res.results[c]["out"]
        for q in range(B):
            out[2048 * q + 256 * c : 2048 * q + 256 * (c + 1)] = o[256 * q : 256 * (q + 1)]
    return out.reshape(B, T, C)



# revision 44
# speedup vs baseline: 1.3314x; 1.3314x over previous
"""Causal self-attention (B=4, T=2048, C=1024, H=16) on 8 trn2 NeuronCores.

Sharding: 2 heads per core for QKV+attention (tensor-parallel over heads);
AllToAll redistributes per-head attention outputs into per-core row slices
for a row-parallel output projection.

All matmuls run in bf16 (fp32 PSUM accumulation): the cost model streams
bf16 at 1 cycle/row at any width, where fp32r pays 4x below 256-wide.
The A*V matmul is flipped (softmax tile stationary, V moving) so each
causal tile pair costs 65 rows instead of 128; the V projection is also
flipped (x stationary, Wv moving) so V comes out token-major with no PE
transpose. Attention output is sent token-major through the A2A and
transposed on the receive side with the DMA xbar transpose, so no PE
cycles are spent on y layout at all. QKV projection of batch b+1 and the
output projection of earlier batches are interleaved into the ACT-bound
attention stream of batch b to fill the PE's exp-wait gaps.
"""

import math
from contextlib import ExitStack

import numpy as np

NCORES = 8
B, T, C = 4, 2048, 1024
H = 16
D = C // H  # 64
HPC = H // NCORES  # heads per core = 2
BT = B * T  # 8192
ROWS_PER_CORE = BT // NCORES  # 1024
QROWS = T // NCORES  # 256 rows per core per batch
NKT = T // 128  # 16 k-tiles per batch
NQT = NKT  # 16 q-tiles per batch
NEG = -1.0e30
AV_SKEW = 1

_compiled = None


def _build(no_collective=False, debug_dump=False):
    import concourse.tile as tile
    from concourse import bacc, mybir
    from concourse.masks import make_identity, make_lower_triangular

    f32 = mybir.dt.float32
    bf16 = mybir.dt.bfloat16

    nc = bacc.Bacc()

    # ---- DRAM I/O (per-core views; same kernel on all 8 cores) ----
    xt_d = nc.dram_tensor("xt", [C, BT], bf16, kind="ExternalInput")
    wqkv_d = nc.dram_tensor("wqkv", [C, 3 * 128], bf16, kind="ExternalInput")
    bqkv_d = nc.dram_tensor("bqkv", [128, 2], f32, kind="ExternalInput")
    bv_d = nc.dram_tensor("bv", [1, 128], f32, kind="ExternalInput")
    wp_d = nc.dram_tensor("wp", [C, C], bf16, kind="ExternalInput")
    bp_d = nc.dram_tensor("bp", [1, C], f32, kind="ExternalInput")
    out_d = nc.dram_tensor("out", [ROWS_PER_CORE, C], f32, kind="ExternalOutput")
    dbg_yl = dbg_ya = dbg_qkv = None
    if debug_dump:
        dbg_yl = nc.dram_tensor("dbg_yl", [B, NCORES, QROWS, 128], f32, kind="ExternalOutput")
        dbg_ya = nc.dram_tensor("dbg_ya", [B, NCORES, QROWS, 128], f32, kind="ExternalOutput")
        dbg_qkv = nc.dram_tensor("dbg_qkv", [3, 128, T], f32, kind="ExternalOutput")

    # internal DRAM for the collectives; y is TOKEN-major: [dst, row, feat]
    y_loc = [nc.dram_tensor(f"y_loc{q}", [NCORES, QROWS, 128], bf16) for q in range(B)]
    y_all = [nc.dram_tensor(f"y_all{q}", [NCORES, QROWS, 128], bf16) for q in range(B)]

    xt_r = xt_d[:, :].rearrange("(j p) t -> p j t", p=128)  # [128, 8, BT]
    wqkv_r = wqkv_d[:, :].rearrange("(j p) f -> p j f", p=128)  # [128, 8, 384]
    wp_r = wp_d[:, :].rearrange("(j p) f -> p j f", p=128)  # [128, 8, 1024]
    # y store view: row = qh*128 + p, feat = s*64 + f
    yl_r = [
        y_loc[q][:, :, :].rearrange("h (qh p) (s f) -> p h qh s f", p=128, s=HPC)
        for q in range(B)
    ]

    with tile.TileContext(nc) as tc, ExitStack() as ctx:
        wpool = ctx.enter_context(tc.tile_pool(name="wpool", bufs=1))
        qkv_pool = ctx.enter_context(tc.tile_pool(name="qkv_pool", bufs=2))
        xt_pool = ctx.enter_context(tc.tile_pool(name="xt_pool", bufs=3))
        pt_pool = ctx.enter_context(tc.tile_pool(name="pt", bufs=NKT + 2))
        r_pool = ctx.enter_context(tc.tile_pool(name="rp", bufs=2))
        y2_pool = ctx.enter_context(tc.tile_pool(name="y2", bufs=2))
        ya_pool = ctx.enter_context(tc.tile_pool(name="ya", bufs=4))
        out_pool = ctx.enter_context(tc.tile_pool(name="op", bufs=2))
        ps_big = ctx.enter_context(tc.tile_pool(name="ps_big", bufs=3, space="PSUM"))
        ps_av = ctx.enter_context(tc.tile_pool(name="ps_av", bufs=2, space="PSUM"))

        # ---- weights + constants ----
        wq_sb = wpool.tile([128, 8, 3 * 128], bf16)
        bias_sb = wpool.tile([128, 2], f32)
        vb_row = wpool.tile([128, 128], f32, tag="vb_row")
        vbias_bc = wpool.tile([128, 128], f32, tag="vbias_bc")
        ident_bf = wpool.tile([128, 128], bf16, tag="ident_bf")
        mnegm = wpool.tile([128, 128], bf16, tag="mnegm")
        wp_sb = wpool.tile([128, 8, C], bf16)
        bp_row = wpool.tile([128, C], f32, tag="bp_row")
        bias_bc = wpool.tile([128, C], f32, tag="bias_bc")

        def emit_wp_load():
            nc.sync.dma_start(out=wp_sb[:, :, :], in_=wp_r[:, :, :])
            nc.sync.dma_start(out=bp_row[0:1, :], in_=bp_d[:, :])
            nc.gpsimd.partition_broadcast(bias_bc[:, :], bp_row[0:1, :])

        result = {}
        ya3_src = (y_loc if no_collective else y_all)[3]

        def phase1(b):
            """QKV projection for batch b (generator with fine-grained
            yields). q,k come out feature-major in qT/kT; v is computed
            flipped (x stationary) so V is token-major: [tok128, kt, s, 65]
            with column 64 reserved for the softmax-denominator ones."""
            qT = qkv_pool.tile([128, T], bf16, tag="qT", name=f"qT{b}")
            kT = qkv_pool.tile([128, T], bf16, tag="kT", name=f"kT{b}")
            V = qkv_pool.tile([128, NKT, HPC, D + 1], bf16, tag="V", name=f"V{b}")
            y2 = y2_pool.tile([128, NQT, HPC, D], bf16, tag="y2", name=f"y2_{b}")
            result[b] = (qT, kT, V, y2)
            nc.vector.memset(V[:, :, :, D : D + 1], 1.0)

            xt_tiles = {}

            def load_xt(tt, eng=None):
                tok0 = b * T + tt * 512
                xt_t = xt_pool.tile([128, 8, 512], bf16, tag="xt", name=f"xt{b}_{tt}")
                (eng or nc.sync).dma_start(
                    out=xt_t[:, :, :], in_=xt_r[:, :, tok0 : tok0 + 512]
                )
                xt_tiles[tt] = xt_t

            if b == 0:
                # startup: interleave per-j weight and x loads so the first
                # accumulation steps can begin as soon as chunk 0 lands
                xt_t = xt_pool.tile([128, 8, 512], bf16, tag="xt", name="xt0_0")
                for jj in range(4):
                    j0 = 2 * jj
                    nc.sync.dma_start(
                        out=wq_sb[:, j0 : j0 + 2, :], in_=wqkv_r[:, j0 : j0 + 2, :]
                    )
                    nc.sync.dma_start(
                        out=xt_t[:, j0 : j0 + 2, :], in_=xt_r[:, j0 : j0 + 2, 0:512]
                    )
                load_xt(1)
                nc.sync.dma_start(out=bias_sb, in_=bqkv_d[:, :])
                nc.sync.dma_start(out=vb_row[0:1, :], in_=bv_d[:, :])
                nc.gpsimd.partition_broadcast(vbias_bc[:, :], vb_row[0:1, :])
                load_xt(2)
                xt_tiles[0] = xt_t
            else:
                load_xt(0)
                load_xt(1)
            for tt in range(4):  # 512-token tiles
                if tt + 2 < 4 and tt + 2 not in xt_tiles:
                    load_xt(tt + 2)
                xt_t = xt_tiles.pop(tt)
                if b == 0 and tt == 0:
                    # j-outer so each arriving (wq_j, xt_j) chunk feeds all
                    # three of q/k/v before PE has to wait for the next one.
                    # Each of the 6 accumulation groups (q, k, 4 v-regions)
                    # gets its own psum bank: only one live group per bank.
                    ps_q = ps_big.tile([128, 1024], f32, tag="big")
                    ps_k = ps_big.tile([128, 1024], f32, tag="big")
                    ps_v = ps_big.tile([128, 1024], f32, tag="big")
                    vdst = [ps_v[:, 0:128], ps_v[:, 512:640],
                            ps_k[:, 512:640], ps_q[:, 512:640]]
                    for j in range(8):
                        for m in range(2):
                            nc.tensor.matmul(
                                (ps_q, ps_k)[m][:, 0:512],
                                wq_sb[:, j, m * 128 : (m + 1) * 128],
                                xt_t[:, j, :],
                                start=(j == 0),
                                stop=(j == 7),
                            )
                        for i in range(4):
                            nc.tensor.matmul(
                                vdst[i],
                                xt_t[:, j, i * 128 : (i + 1) * 128],
                                wq_sb[:, j, 2 * 128 : 3 * 128],
                                start=(j == 0),
                                stop=(j == 7),
                            )
                        yield
                    nc.vector.tensor_scalar_add(
                        qT[:, 0:512], ps_q[:, 0:512], bias_sb[:, 0:1]
                    )
                    nc.vector.tensor_scalar_add(
                        kT[:, 0:512], ps_k[:, 0:512], bias_sb[:, 1:2]
                    )
                    yield
                    for i in range(4):
                        nc.vector.tensor_add(
                            V[:, i, :, 0:D],
                            vdst[i].rearrange("p (s f) -> p s f", s=HPC),
                            vbias_bc[:, :].rearrange("p (s f) -> p s f", s=HPC),
                        )
                    yield
                    continue
                for m in range(2):  # q, k feature chunks
                    ps = ps_big.tile([128, 1024], f32, tag="big")
                    for j in range(8):
                        nc.tensor.matmul(
                            ps[:, 0:512],
                            wq_sb[:, j, m * 128 : (m + 1) * 128],
                            xt_t[:, j, :],
                            start=(j == 0),
                            stop=(j == 7),
                        )
                        if j % 2 == 1:
                            yield
                    dst = qT if m == 0 else kT
                    nc.vector.tensor_scalar_add(
                        dst[:, tt * 512 : (tt + 1) * 512], ps[:, 0:512], bias_sb[:, m : m + 1]
                    )
                    yield
                # v chunk: flipped matmul -> token-major [tok, 2*64]
                ps = ps_big.tile([128, 1024], f32, tag="big")
                for i in range(4):
                    kt = tt * 4 + i
                    for j in range(8):
                        nc.tensor.matmul(
                            ps[:, i * 128 : (i + 1) * 128],
                            xt_t[:, j, i * 128 : (i + 1) * 128],
                            wq_sb[:, j, 2 * 128 : 3 * 128],
                            start=(j == 0),
                            stop=(j == 7),
                        )
                        if j % 4 == 3:
                            yield
                    nc.vector.tensor_add(
                        V[:, kt, :, 0:D],
                        ps[:, i * 128 : (i + 1) * 128].rearrange("p (s f) -> p s f", s=HPC),
                        vbias_bc[:, :].rearrange("p (s f) -> p s f", s=HPC),
                    )
                yield

        def phase2qs(b, s, qs):
            """Causal attention for (batch b, head-slot s, query supertile
            qs). QK^T scores per k-strip, exp on ACT, then the flipped AV
            accumulation (p stationary, V moving, 65-wide) with AV_SKEW
            strips of skew."""
            qT, kT, V, y2 = result[b]
            p0 = s * D
            if True:
                nkt = 8 * (qs + 1)
                pts = {}
                psqs = {}

                def emit_part1(qt):
                    """Bulk of a qtile's AV accumulation group (strips
                    0..g-1). One live group per psum bank; the single
                    finisher matmul and the normalization run in
                    emit_part2 once strip g's exp is out."""
                    g = qs * 8 + qt
                    psq = ps_av.tile([128, 512], f32, tag="avq", name=f"avq{b}{s}{g}")
                    psqs[qt] = psq
                    for kt in range(g):
                        nc.tensor.matmul(
                            psq[:, 0 : D + 1],
                            pts[kt][:, qt * 128 : (qt + 1) * 128],
                            V[:, kt, s, :],
                            start=(kt == 0),
                            stop=False,
                        )

                def emit_part2(qt):
                    g = qs * 8 + qt
                    psq = psqs.pop(qt)
                    nc.tensor.matmul(
                        psq[:, 0 : D + 1],
                        pts[g][:, qt * 128 : (qt + 1) * 128],
                        V[:, g, s, :],
                        start=(g == 0),
                        stop=True,
                    )
                    r_t = r_pool.tile([128, 1], f32, tag="r")
                    nc.vector.reciprocal(r_t[:, :], psq[:, D : D + 1])
                    nc.vector.tensor_scalar_mul(
                        y2[:, g, s, :], psq[:, 0:D], r_t[:, :]
                    )
                    # stores (token-major y to DRAM). Batches 0-2: one merged
                    # store per supertile once slot 1 is done. Batch 3 slot 1:
                    # eager stores + ya transposes so the projection of the
                    # last batch can start as early as possible.
                    if b == 3 and s == 1 and qs == 1 and qt in (3, 7):
                        h0 = 4 + 2 * (qt // 4)
                        nc.sync.dma_start(
                            out=yl_r[b][:, h0 : h0 + 2, :, s, :],
                            in_=y2[:, 2 * h0 : 2 * h0 + 4, s, :],
                        )
                    elif qt == 7:
                        if b < 3:
                            if s == 1:
                                nc.sync.dma_start(
                                    out=yl_r[b][:, qs * 4 : (qs + 1) * 4, :, :, :],
                                    in_=y2[:, qs * 8 : (qs + 1) * 8, :, :],
                                )
                        else:
                            nc.sync.dma_start(
                                out=yl_r[b][:, qs * 4 : (qs + 1) * 4, :, s, :],
                                in_=y2[:, qs * 8 : (qs + 1) * 8, s, :],
                            )

                for kt in range(nkt):
                    off = max(0, kt * 128 - qs * 1024)
                    segs = [(off, 512), (512, 1024)] if off < 512 else [(off, 1024)]
                    diag = kt * 128 >= qs * 1024
                    ps_s = ps_big.tile([128, 1024], f32, tag="big")
                    for lo, hi in segs:
                        nc.tensor.matmul(
                            ps_s[:, lo:hi],
                            kT[p0 : p0 + D, kt * 128 : (kt + 1) * 128],
                            qT[p0 : p0 + D, qs * 1024 + lo : qs * 1024 + hi],
                            start=True,
                            # the diagonal segment's group stays open: the
                            # causal-mask accumulate below closes it, so the
                            # exp's dependency covers the mask as well
                            stop=not (diag and lo == off),
                        )
                    if diag:
                        nc.tensor.matmul(
                            ps_s[:, off : off + 128],
                            ident_bf[:, :],
                            mnegm[:, :],
                            start=False,
                            stop=True,
                        )
                    pt = pt_pool.tile([128, 1024], bf16)
                    nc.scalar.activation(
                        pt[:, off:1024],
                        ps_s[:, off:1024],
                        mybir.ActivationFunctionType.Exp,
                        scale=1.0 / math.sqrt(D),
                    )
                    pts[kt] = pt
                    qt_b = kt - qs * 8 - AV_SKEW
                    if 0 <= qt_b < 8:
                        emit_part2(qt_b)
                    qt_a = kt - qs * 8
                    if 0 <= qt_a < 8:
                        emit_part1(qt_a)
                    yield
                for qt in range(max(0, nkt - qs * 8 - AV_SKEW), 8):
                    emit_part2(qt)
                    yield

        def emit_a2a(q):
            if no_collective:
                return
            nc.gpsimd.collective_compute(
                "AllToAll",
                mybir.AluOpType.bypass,
                replica_groups=[list(range(NCORES))],
                ins=[y_loc[q][:, :, :]],
                outs=[y_all[q][:, :, :]],
            )

        def proj(q, ya=None):
            """Output projection for this core's 256 rows of batch q.
            ya arrives feature-major via the DMA xbar transpose."""
            y_src = y_loc[q] if no_collective else y_all[q]
            if ya is None:
                ya = ya_pool.tile([128, 8, QROWS], bf16)
                # data has long been ready: one big xbar transpose
                nc.sync.dma_start_transpose(
                    ya[:, :, :].rearrange("p h r -> p (h r)"),
                    y_src[:, :, :].rearrange("h r f -> (h r) f"),
                )
            for rt in range(2):
                ps_o = ps_big.tile([128, 1024], f32, tag="big")
                out_sb = out_pool.tile([128, 1024], f32)
                row = q * 256 + rt * 128
                # emit per 512-wide half so the bias-add + store of half 0
                # overlaps the matmuls of half 1
                for lo, hi in ((0, 512), (512, 1024)):
                    for i in range(8):
                        nc.tensor.matmul(
                            ps_o[:, lo:hi],
                            ya[:, i, rt * 128 : (rt + 1) * 128],
                            wp_sb[:, i, lo:hi],
                            start=(i == 0),
                            stop=(i == 7),
                        )
                        if i % 2 == 1:
                            yield
                    nc.vector.tensor_add(
                        out_sb[:, lo:hi], ps_o[:, lo:hi], bias_bc[:, lo:hi]
                    )
                    nc.gpsimd.dma_start(
                        out=out_d[row : row + 128, lo:hi], in_=out_sb[:, lo:hi]
                    )
                    yield

        def proj_last(q, ya_parts):
            """Projection of the final batch: accumulate the early shards
            first across both row-tiles, so only the i=6,7 contributions
            (whose ya lands last) sit on the critical tail. ya arrives in
            three independently-tracked tiles (shards 0-3, 4-5, 6-7) so an
            early matmul never waits on a later transpose."""
            def ya_at(i):
                if i < 4:
                    return ya_parts[0][:, i, :]
                if i < 6:
                    return ya_parts[1][:, i - 4, :]
                return ya_parts[2][:, i - 6, :]

            ps_os = [
                ps_big.tile([128, 1024], f32, tag="big", name=f"ps_o{q}_{rt}")
                for rt in range(2)
            ]
            for i in range(6):
                for rt in range(2):
                    for lo, hi in ((0, 512), (512, 1024)):
                        nc.tensor.matmul(
                            ps_os[rt][:, lo:hi],
                            ya_at(i)[:, rt * 128 : (rt + 1) * 128],
                            wp_sb[:, i, lo:hi],
                            start=(i == 0),
                            stop=False,
                        )
                yield
            for rt in range(2):
                out_sb = out_pool.tile([128, 1024], f32)
                row = q * 256 + rt * 128
                for lo, hi in ((0, 512), (512, 1024)):
                    for i in (6, 7):
                        nc.tensor.matmul(
                            ps_os[rt][:, lo:hi],
                            ya_at(i)[:, rt * 128 : (rt + 1) * 128],
                            wp_sb[:, i, lo:hi],
                            start=False,
                            stop=(i == 7),
                        )
                    nc.vector.tensor_add(
                        out_sb[:, lo:hi], ps_os[rt][:, lo:hi], bias_bc[:, lo:hi]
                    )
                    nc.sync.dma_start(
                        out=out_d[row : row + 128, lo:hi], in_=out_sb[:, lo:hi]
                    )
                    yield

        def phase2(b, s):
            yield from phase2qs(b, s, 0)
            yield from phase2qs(b, s, 1)

        def run_interleaved(primary, filler, steps=1.5):
            credit = 0.0
            for _ in primary:
                credit += steps
                while filler is not None and credit >= 1.0:
                    credit -= 1.0
                    try:
                        next(filler)
                    except StopIteration:
                        filler = None
            return filler

        def drain(gen):
            if gen is not None:
                for _ in gen:
                    pass

        def chain(*gens):
            for g in gens:
                if g is not None:
                    yield from g

        # startup: pump batch 0 qkv through its first two token tiles, then
        # interleave the (already satisfiable) first attention supertile into
        # the DMA-paced remainder; constants that phase1 itself doesn't need
        # are emitted after its first group
        p10 = phase1(0)
        next(p10)
        make_identity(nc, ident_bf[:, :])
        make_lower_triangular(nc, mnegm[:, :], val=NEG, diag=False)
        NY01 = 28  # yields to finish token tiles 0 and 1 of batch 0
        for _ in range(NY01):
            next(p10)
        p10 = run_interleaved(phase2qs(0, 0, 0), p10, steps=4.0)
        drain(p10)
        filler = phase1(1)
        filler = run_interleaved(phase2qs(0, 0, 1), filler)
        filler = run_interleaved(phase2(0, 1), filler)
        drain(filler)
        emit_wp_load()  # off the critical startup path
        emit_a2a(0)
        for b in range(1, B):
            parts = []
            if b < B - 1:
                parts.append(phase1(b + 1))
            ya3 = None
            if b == 3:
                parts.append(proj(0))
                parts.append(proj(1))
                parts.append(proj(2))
                ya3 = [
                    ya_pool.tile([128, 4, QROWS], bf16, tag="ya3a", name="ya3a"),
                    ya_pool.tile([128, 2, QROWS], bf16, tag="ya3b", name="ya3b"),
                    ya_pool.tile([128, 2, QROWS], bf16, tag="ya3c", name="ya3c"),
                ]
            steps = 0.85 if b == 3 else 1.2
            filler = chain(*parts) if parts else None
            filler = run_interleaved(phase2(b, 0), filler, steps=steps)
            filler = run_interleaved(phase2(b, 1), filler, steps=steps)
            drain(filler)
            emit_a2a(b)
        # ya3 transposes must be emitted AFTER the collective that writes
        # y_all[3]: the tile framework cannot express a dependency on a
        # writer that has not been emitted yet (use-before-def)
        nc.sync.dma_start_transpose(
            ya3[0][:, :, :].rearrange("p h r -> p (h r)"),
            ya3_src[0:4, :, :].rearrange("h r f -> (h r) f"),
        )
        for k, hh in ((1, 4), (2, 6)):
            nc.sync.dma_start_transpose(
                ya3[k][:, :, :].rearrange("p h r -> p (h r)"),
                ya3_src[hh : hh + 2, :, :].rearrange("h r f -> (h r) f"),
            )
        drain(proj_last(3, ya3))
        if debug_dump:
            dpool = ctx.enter_context(tc.tile_pool(name="dbg", bufs=1))
            for bq in range(B):
                for i in range(NCORES):
                    t = dpool.tile([128, 2, 128], bf16, tag="da", name=f"da{bq}_{i}")
                    nc.sync.dma_start(out=t[:, :, :], in_=y_loc[bq][i, :, :].rearrange("(a p) f -> p a f", p=128))
                    t2 = dpool.tile([128, 2, 128], f32, tag="db", name=f"db{bq}_{i}")
                    nc.vector.tensor_copy(t2[:, :, :], t[:, :, :])
                    nc.sync.dma_start(out=dbg_yl[bq, i, :, :].rearrange("(a p) f -> p a f", p=128), in_=t2[:, :, :])
                    u = dpool.tile([128, 2, 128], bf16, tag="dc", name=f"dc{bq}_{i}")
                    nc.sync.dma_start(out=u[:, :, :], in_=y_all[bq][i, :, :].rearrange("(a p) f -> p a f", p=128))
                    u2 = dpool.tile([128, 2, 128], f32, tag="dd", name=f"dd{bq}_{i}")
                    nc.vector.tensor_copy(u2[:, :, :], u[:, :, :])
                    nc.sync.dma_start(out=dbg_ya[bq, i, :, :].rearrange("(a p) f -> p a f", p=128), in_=u2[:, :, :])
            qT3, kT3, V3, _ = result[3]
            for idx, tt in enumerate((qT3, kT3)):
                f = dpool.tile([128, T], f32, tag="de", name=f"de{idx}")
                nc.vector.tensor_copy(f[:, :], tt[:, :])
                nc.sync.dma_start(out=dbg_qkv[idx, :, :], in_=f[:, :])
            vf = dpool.tile([128, T], f32, tag="dv")
            nc.vector.tensor_copy(
                vf[:, :].rearrange("p (kt s d) -> p kt s d", kt=NKT, s=HPC),
                V3[:, :, :, 0:D],
            )
            nc.sync.dma_start(out=dbg_qkv[2, :, :], in_=vf[:, :])

    nc.compile()
    return nc


def _get_compiled():
    global _compiled
    if _compiled is None:
        _compiled = _build()
    return _compiled


def _make_in_maps(x, W_attn, b_attn, W_proj, b_proj):
    import ml_dtypes

    bf = ml_dtypes.bfloat16
    xt = np.ascontiguousarray(x.reshape(BT, C).T.astype(bf))
    wp = np.ascontiguousarray(W_proj.astype(bf))
    bp = np.ascontiguousarray(b_proj.reshape(1, C).astype(np.float32))
    in_maps = []
    for c in range(NCORES):
        heads = [HPC * c + s for s in range(HPC)]
        cols = []
        for m in range(3):  # q, k, v blocks of W_attn
            for h in heads:
                cols.extend(range(m * C + h * D, m * C + (h + 1) * D))
        cols = np.asarray(cols)
        in_maps.append(
            {
                "xt": xt,
                "wqkv": np.ascontiguousarray(W_attn[:, cols].astype(bf)),
                "bqkv": np.ascontiguousarray(
                    b_attn[cols[0:256]].reshape(2, 128).T.astype(np.float32)
                ),
                "bv": np.ascontiguousarray(
                    b_attn[cols[256:384]].reshape(1, 128).astype(np.float32)
                ),
                "wp": wp,
                "bp": bp,
            }
        )
    return in_maps


def kernel(x, W_attn, b_attn, W_proj, b_proj):
    from concourse.bass_utils import run_bass_kernel_spmd

    x = np.asarray(x, dtype=np.float32)
    W_attn = np.asarray(W_attn, dtype=np.float32)
    b_attn = np.asarray(b_attn, dtype=np.float32)
    W_proj = np.asarray(W_proj, dtype=np.float32)
    b_proj = np.asarray(b_proj, dtype=np.float32)

    nc = _get_compiled()
    in_maps = _make_in_maps(x, W_attn, b_attn, W_proj, b_proj)
    res = run_bass_kernel_spmd(nc, in_maps, core_ids=list(range(NCORES)))

    # core c's output: for each batch q, rows [256c, 256c+256) of that batch
    out = np.empty((BT, C), dtype=np.float32)
    for c in range(NCORES):
        o = res.results[c]["out"]
        for q in range(B):
            out[2048 * q + 256 * c : 2048 * q + 256 * (c + 1)] = o[256 * q : 256 * (q + 1)]
    return out.reshape(B, T, C)
